# revision 19
# baseline (speedup 1.0000x reference)
"""Self-contained Trainium2 Bass kernel for nn_Attention (B=8, N=1024, C=1024, H=16, D=64).

Sharding: data-parallel over batch B across the 8 NeuronCores (one batch element
per core, no collectives). Per-core program (bf16 matmuls, fp32 accumulate):

  - Weights are packed ON HOST (once; device-cached by content hash) into bf16
    arrays whose DRAM layout matches the SBUF tiles exactly, so every weight
    DMA is 128 x contiguous-4KB+ descriptors and needs no on-chip cast.
  - x is PE-transposed to xT [C, N]. Token positions use a fixed device
    ordering q = i*128+p  <->  DRAM row n = 8p+i so the x load is 128 x 32KB
    contiguous descriptors; attention is permutation-invariant along tokens and
    the output DMA inverts the ordering when writing DRAM.
  - qkT[c',n] (transposed q/k) is computed per head-pair and interleaved with
    the attention pipeline so the TensorEngine fills the gaps while the scalar
    engine (ACT) streams the softmax exps.
  - Scores are computed transposed, sT[k,q] = kT.T @ qT, with two heads packed
    onto the PE array via tile_position row groups. p = exp(sT*scale) on ACT
    (bf16, no max-subtraction: scores are O(5) so exp cannot overflow).
  - v carries an appended ones column (v_ext), so oT_ext = v_ext.T @ p also
    emits the softmax denominators as row D. Normalization: the denominator row
    is DMA-reshaped to [8,128] so the DVE serial divide runs 8 lanes wide, then
    partition-broadcast via a DRAM bounce (DMA only), then one DVE multiply.
  - out[n, c'] = o_catT.T @ w_proj + bias.
"""

import numpy as np

B = 8
N = 1024          # tokens
C = 1024          # model dim
H = 16            # heads
D = 64            # head dim
SCALE = D ** -0.5
NT = N // 128     # token tiles
CT = C // 128     # channel tiles
HP = H // 2       # head pairs

_CACHE: dict = {}


def pack_weights(w_qkv, w_proj):
    """Host-side weight packing into bf16 DRAM arrays laid out exactly like
    the SBUF tiles they are DMA'd into (contiguous per partition)."""
    import ml_dtypes

    bf16 = ml_dtypes.bfloat16
    w_qkv = np.asarray(w_qkv, np.float32)
    w_proj = np.asarray(w_proj, np.float32)
    # wqk_pack[p, pj, kt, j] = w_qkv[kt*128+p, pj*128+j] (j<128: q; else k)
    wq = w_qkv[:, 0:C].reshape(CT, 128, HP, 128).transpose(1, 2, 0, 3)
    wk = w_qkv[:, C : 2 * C].reshape(CT, 128, HP, 128).transpose(1, 2, 0, 3)
    wqk_pack = np.ascontiguousarray(
        np.concatenate([wq, wk], axis=3), dtype=bf16
    )  # [128, HP, CT, 256]
    # wv_pack[p, ch, kt, j] = w_qkv[kt*128+p, 2C + ch*512 + j]
    wv_pack = np.ascontiguousarray(
        w_qkv[:, 2 * C :].reshape(CT, 128, 2, 512).transpose(1, 2, 0, 3), dtype=bf16
    )  # [128, 2, CT, 512]
    # wp_pack[p, kt, :] = w_proj[kt*128+p, :]
    wp_pack = np.ascontiguousarray(
        w_proj.reshape(CT, 128, C).transpose(1, 0, 2), dtype=bf16
    )  # [128, CT, C]
    return wqk_pack, wv_pack, wp_pack


def make_in_maps(x, w_qkv, b_qkv, w_proj, b_proj):
    x = np.ascontiguousarray(np.asarray(x, dtype=np.float32))
    wqk_pack, wv_pack, wp_pack = pack_weights(w_qkv, w_proj)
    shared = {
        "wqk_pack": wqk_pack,
        "wv_pack": wv_pack,
        "wp_pack": wp_pack,
        "b_qkv": np.ascontiguousarray(np.asarray(b_qkv, dtype=np.float32)),
        "b_proj": np.ascontiguousarray(np.asarray(b_proj, dtype=np.float32)),
    }
    return [{"x": x[b], **shared} for b in range(B)]


def _build_program(repeat: int = 1, max_phase: int = 3):
    import concourse.mybir as mybir
    import concourse.tile as tile
    from concourse import bacc
    from concourse.masks import make_identity
    import concourse.bass as bass

    F32 = mybir.dt.float32
    BF16 = mybir.dt.bfloat16
    AF = mybir.ActivationFunctionType

    nc = bacc.Bacc("TRN2", target_bir_lowering=False, debug=False, num_devices=B)

    x_ext = nc.declare_dram_parameter("x", [N, C], F32, isOutput=False)
    wqk_ext = nc.declare_dram_parameter("wqk_pack", [128, HP, CT, 256], BF16, isOutput=False)
    wv_ext = nc.declare_dram_parameter("wv_pack", [128, 2, CT, 512], BF16, isOutput=False)
    wp_ext = nc.declare_dram_parameter("wp_pack", [128, CT, C], BF16, isOutput=False)
    bqkv_ext = nc.declare_dram_parameter("b_qkv", [3 * C], F32, isOutput=False)
    bproj_ext = nc.declare_dram_parameter("b_proj", [C], F32, isOutput=False)
    out_ext = nc.declare_dram_parameter("out", [N, C], F32, isOutput=True)

    x_ap = x_ext.ap()
    wqk_ap = wqk_ext.ap()
    wv_ap = wv_ext.ap()
    wp_ap = wp_ext.ap()
    bqkv_ap = bqkv_ext.ap()
    bproj_ap = bproj_ext.ap()
    out_ap = out_ext.ap()

    def bcast_part(src_ap, parts):
        return bass.AP(
            tensor=src_ap.tensor,
            offset=src_ap.offset,
            ap=[[0, parts]] + [list(p) for p in src_ap.ap[1:]],
        )

    def bcast_row(src_1d_ap, parts):
        return bass.AP(
            tensor=src_1d_ap.tensor,
            offset=src_1d_ap.offset,
            ap=[[0, parts]] + [list(p) for p in src_1d_ap.ap],
        )

    with tile.TileContext(nc) as tc:
        # ---- persistent SBUF ----
        identity, _free_id = tc.tile([128, 128], F32, name="identity")
        make_identity(nc, identity)

        v_ext, _free_vext = tc.tile([128, NT, H, D + 1], BF16, name="v_ext")
        nc.vector.memset(v_ext[:, :, :, D : D + 1], 1.0)
        bq_pp, _free_bq = tc.tile([128, 2 * CT], F32, name="bq_pp")
        bv_bc, _free_bv = tc.tile([128, C], BF16, name="bv_bc")
        bp_bc, _free_bp = tc.tile([128, C], BF16, name="bp_bc")

        nc.sync.dma_start(
            out=bq_pp, in_=bqkv_ap[0 : 2 * C].rearrange("(t p) -> p t", p=128)
        )
        nc.gpsimd.dma_start(out=bv_bc, in_=bcast_row(bqkv_ap[2 * C : 3 * C], 128))
        nc.gpsimd.dma_start(out=bp_bc, in_=bcast_row(bproj_ap, 128))

        for rep in range(repeat):
            s = f"r{rep}_"

            o_catT, free_ocat = tc.tile([128, CT, N], BF16, name=s + "o_catT")
            wproj, free_wproj = tc.tile([128, CT, C], BF16, name=s + "wproj")
            xT, free_xT = tc.tile([128, CT, N], BF16, name=s + "xT")
            wv_bf, free_wv = tc.tile([128, CT, C], BF16, name=s + "wv_bf")

            # ================= phase 0: x -> xT =================
            # Device token order q = i*128 + p reads DRAM row n = 8p + i, so
            # each partition p loads rows 8p..8p+7: one 32KB contiguous run.
            with (
                tc.tile_pool(name=s + "x_pool", bufs=1) as x_pool,
                tc.tile_pool(name=s + "pt_pool", bufs=4, space="PSUM") as pt_pool,
            ):
                x_all = x_pool.tile(
                    [128, NT, C], F32, name=f"{s}x_all", tag="x_all"
                )
                x_src = x_ap.rearrange("(p i) c -> p i c", i=NT)
                HN = NT // 2
                for half in range(2):
                    isl = slice(half * HN, (half + 1) * HN)
                    nc.sync.dma_start(out=x_all[:, isl, :], in_=x_src[:, isl, :])
                    for j in range(CT):
                        for i in range(half * HN, (half + 1) * HN):
                            ps_t = pt_pool.tile(
                                [128, 128], F32, name=f"{s}ps_t{i}_{j}", tag="ps_t"
                            )
                            nc.tensor.transpose(
                                ps_t, x_all[:, i, j * 128 : (j + 1) * 128], identity
                            )
                            nc.vector.tensor_copy(
                                out=xT[:, j, i * 128 : (i + 1) * 128], in_=ps_t
                            )

            if max_phase == 0:
                nc.gpsimd.dma_start(out=out_ap[0:128, :], in_=xT[:, 0, :])
                free_wv()
                free_xT()
                free_wproj()
                free_ocat()
                continue

            # ---- interleaved main body ----
            with (
                tc.tile_pool(name=s + "wqk_pool", bufs=2) as wqk_pool,
                tc.tile_pool(name=s + "qk_pool", bufs=4) as qk_pool,
                tc.tile_pool(name=s + "ps_big", bufs=2, space="PSUM") as ps_big_pool,
                tc.tile_pool(name=s + "ps_o", bufs=1, space="PSUM") as ps_o_pool,
                tc.tile_pool(name=s + "ps_v", bufs=2, space="PSUM") as ps_v_pool,
                tc.tile_pool(name=s + "pT_pool", bufs=10) as pT_pool,
                tc.tile_pool(name=s + "l_pool", bufs=2) as l_pool,
                tc.tile_pool(name=s + "l_dram", bufs=2, space="DRAM") as l_dram_pool,
            ):

                def emit_wqk_strip(pj):
                    """Load the packed [C, 128|128] q/k column strips for pair pj."""
                    strip = wqk_pool.tile(
                        [128, CT, 256], BF16, name=f"{s}wqk{pj}", tag="wqk"
                    )
                    nc.scalar.dma_start(out=strip, in_=wqk_ap[:, pj, :, :])
                    return strip

                def emit_wv_chunk(ch):
                    nc.scalar.dma_start(
                        out=wv_bf[:, :, ch * 512 : ch * 512 + 512],
                        in_=wv_ap[:, ch, :, :],
                    )

                def emit_qk_pair(pj, strip):
                    """qT/kT for head pair pj, chunk-major so the pair's first
                    score matmuls unblock after half the projection work."""
                    tiles = [
                        qk_pool.tile([128, N], BF16, name=f"{s}qk{pj}_{qk}", tag="qk")
                        for qk in range(2)
                    ]
                    for ch in range(2):
                        nsl = slice(ch * 512, ch * 512 + 512)
                        for qk in range(2):  # 0 = q, 1 = k
                            jj = qk * CT + pj
                            ps1 = ps_v_pool.tile(
                                [128, 512], F32, name=f"{s}ps1_{jj}_{ch}", tag="ps_v"
                            )
                            for kt in range(CT):
                                nc.tensor.matmul(
                                    ps1,
                                    strip[:, kt, qk * 128 : qk * 128 + 128],
                                    xT[:, kt, nsl],
                                    start=(kt == 0),
                                    stop=(kt == CT - 1),
                                )
                            nc.vector.tensor_scalar_add(
                                out=tiles[qk][:, nsl], in0=ps1,
                                scalar1=bq_pp[:, jj : jj + 1],
                            )
                    return tiles

                def make_qk_emitter(pj, strip):
                    """Sliced emission of pair pj's q/k projection: 2 MMs per
                    call (16 calls = 4 chains of 8), so score/pv matmuls never
                    sit behind a 32-matmul projection block in the PE queue."""
                    tiles = [
                        qk_pool.tile([128, N], BF16, name=f"{s}qk{pj}_{qk}", tag="qk")
                        for qk in range(2)
                    ]
                    state = {"c": 0, "k": 0, "ps1": None}

                    def emit_slice():
                        if state["c"] >= 4:
                            return
                        ch, qk = state["c"] // 2, state["c"] % 2
                        nsl = slice(ch * 512, ch * 512 + 512)
                        jj = qk * CT + pj
                        if state["k"] == 0:
                            state["ps1"] = ps_v_pool.tile(
                                [128, 512], F32, name=f"{s}ps1_{jj}_{ch}", tag="ps_v"
                            )
                        for kt in (state["k"], state["k"] + 1):
                            nc.tensor.matmul(
                                state["ps1"],
                                strip[:, kt, qk * 128 : qk * 128 + 128],
                                xT[:, kt, nsl],
                                start=(kt == 0),
                                stop=(kt == CT - 1),
                            )
                        state["k"] += 2
                        if state["k"] == CT:
                            nc.vector.tensor_scalar_add(
                                out=tiles[qk][:, nsl], in0=state["ps1"],
                                scalar1=bq_pp[:, jj : jj + 1],
                            )
                            state["k"] = 0
                            state["c"] += 1

                    return tiles, emit_slice

                def emit_v_chunk(ch):
                    for m in range(NT):
                        ps_v = ps_v_pool.tile(
                            [128, 512], F32, name=f"{s}ps_vv{m}_{ch}", tag="ps_v"
                        )
                        for kt in range(CT):
                            nc.tensor.matmul(
                                ps_v,
                                xT[:, kt, m * 128 : (m + 1) * 128],
                                wv_bf[:, kt, ch * 512 : ch * 512 + 512],
                                start=(kt == 0),
                                stop=(kt == CT - 1),
                            )
                        nc.vector.tensor_add(
                            out=v_ext[:, m, ch * 8 : ch * 8 + 8, 0:D],
                            in0=ps_v.rearrange("p (h d) -> p h d", d=D),
                            in1=bv_bc[:, ch * 512 : ch * 512 + 512].rearrange(
                                "p (h d) -> p h d", d=D
                            ),
                        )

                def emit_attn_head(h, qp, kp, extra=None):
                    pj, hh = h // 2, h % 2
                    hb = hh * 64
                    ps_o = ps_o_pool.tile(
                        [D + 1, N], F32, name=f"{s}ps_o{h}", tag="ps_o"
                    )
                    for kt in range(NT):
                        ksl = slice(kt * 128, (kt + 1) * 128)
                        # The scores->exp->pv chain paces the whole attention
                        # phase (ACT is the bottleneck engine): keep it ahead
                        # of the filler qk-projection slices in the scheduler's
                        # priority order, or exps stall behind hoisted filler.
                        with tc.high_priority(offset=50000):
                            ps_sc = ps_big_pool.tile(
                                [128, N], F32, name=f"{s}ps_sc{h}_{kt}", tag="ps_big"
                            )
                            for ch in range(2):
                                nsl = slice(ch * 512, ch * 512 + 512)
                                nc.tensor.matmul(
                                    ps_sc[:, nsl],
                                    kp[hb : hb + 64, ksl],
                                    qp[hb : hb + 64, nsl],
                                    start=True,
                                    stop=True,
                                    tile_position=(hb, 0),
                                )
                            pT = pT_pool.tile(
                                [128, N], BF16, name=f"{s}pT{h}_{kt}", tag="pT"
                            )
                            nc.scalar.activation(
                                out=pT, in_=ps_sc, func=AF.Exp, scale=SCALE
                            )
                            for ch in range(2):
                                nsl = slice(ch * 512, ch * 512 + 512)
                                nc.tensor.matmul(
                                    ps_o[:, nsl],
                                    v_ext[:, kt, h, :],
                                    pT[:, nsl],
                                    start=(kt == 0),
                                    stop=(kt == NT - 1),
                                )
                        if extra is not None:
                            extra()
                    # drain PSUM fast (frees the bank for the next head's pv),
                    # then normalize from SBUF off the critical path
                    o_raw = l_pool.tile(
                        [D + 1, N], F32, name=f"{s}o_raw{h}", tag="o_raw", bufs=2
                    )
                    nc.vector.tensor_copy(out=o_raw, in_=ps_o)
                    # 1/l on 8 partitions (DVE divide is 8 cyc/elem, so shrink
                    # the per-lane free dim), then partition-broadcast via a
                    # DRAM bounce (DMA-only, no GPSIMD).
                    l_rs = l_pool.tile([8, N // 8], F32, name=f"{s}l_rs{h}", tag="l_rs")
                    nc.scalar.dma_start(out=l_rs, in_=o_raw[D : D + 1, :])
                    l_inv8 = l_pool.tile([8, N // 8], BF16, name=f"{s}l_inv8{h}", tag="l_inv8")
                    with nc.allow_low_precision(reason="1/l in bf16 is ample for 2e-2 tol"):
                        nc.vector.reciprocal(out=l_inv8, in_=l_rs)
                    ld = l_dram_pool.tile([1, N], BF16, name=f"{s}ld{h}", tag="ld")
                    nc.scalar.dma_start(out=ld, in_=l_inv8)
                    l_bc = l_pool.tile([D, N], BF16, name=f"{s}l_bc{h}", tag="l_bc")
                    nc.scalar.dma_start(out=l_bc, in_=bcast_part(ld[0:1, :], D))
                    nc.vector.tensor_mul(
                        out=o_catT[hb : hb + 64, pj, :],
                        in0=o_raw[0:D, :],
                        in1=l_bc,
                    )

                # prologue: pair 0's qk at full speed, then all of v (so wv_bf
                # and the ps_v banks are free for the attention phase)
                strip0 = emit_wqk_strip(0)
                emit_wv_chunk(0)
                emit_wv_chunk(1)
                pair_tiles = emit_qk_pair(0, strip0)
                strip_next = emit_wqk_strip(1)
                emit_v_chunk(0)
                emit_v_chunk(1)
                next_tiles, emit_slice = make_qk_emitter(1, strip_next)
                for h in range(H):
                    pj = h // 2
                    emit_attn_head(h, *pair_tiles, extra=emit_slice)
                    if h % 2 == 1:
                        pair_tiles = next_tiles
                        if pj + 2 < HP:
                            strip_next = emit_wqk_strip(pj + 2)
                            next_tiles, emit_slice = make_qk_emitter(
                                pj + 2, strip_next
                            )
                        else:
                            emit_slice = None
                    if h == 2:
                        # stream w_proj in during attention (packed, no cast)
                        nc.scalar.dma_start(out=wproj, in_=wp_ap)

            free_wv()
            free_xT()

            # ================= projection =================
            # m-tile m holds DRAM rows n = 8p + m: invert the device token
            # ordering in the output DMA access pattern.
            out_dst = out_ap.rearrange("(p i) c -> p i c", i=NT)
            with (
                tc.tile_pool(name=s + "ps_y", bufs=4, space="PSUM") as ps_y_pool,
                tc.tile_pool(name=s + "y_pool", bufs=2) as y_pool,
            ):
                for m in range(NT):
                    y_sb = y_pool.tile([128, C], F32, name=f"{s}y_sb{m}", tag="y_sb")
                    for ch in range(2):
                        nsl = slice(ch * 512, ch * 512 + 512)
                        ps_y = ps_y_pool.tile(
                            [128, 512], F32, name=f"{s}ps_y{m}_{ch}", tag="ps_y"
                        )
                        for j in range(CT):
                            nc.tensor.matmul(
                                ps_y,
                                o_catT[:, j, m * 128 : (m + 1) * 128],
                                wproj[:, j, nsl],
                                start=(j == 0),
                                stop=(j == CT - 1),
                            )
                        nc.vector.tensor_add(
                            out=y_sb[:, nsl], in0=ps_y, in1=bp_bc[:, nsl]
                        )
                    nc.scalar.dma_start(out=out_dst[:, m, :], in_=y_sb)

            free_wproj()
            free_ocat()

        _free_bp()
        _free_bv()
        _free_bq()
        _free_vext()
        _free_id()

    nc.compile()
    return nc


def get_program(repeat: int = 1, max_phase: int = 3):
    key = ("nc", repeat, max_phase)
    if key not in _CACHE:
        _CACHE[key] = _build_program(repeat, max_phase)
    return _CACHE[key]


def _get_runner():
    """Persistent jitted SPMD executor (avoids re-tracing per kernel() call).

    Mirrors concourse.bass2jax.run_bass_via_pjrt's multi-core path, but caches
    the compiled callable so repeat invocations cost only dispatch + transfer,
    and device-caches the (usually unchanged) weight arrays by content hash.
    """
    if "runner" in _CACHE:
        return _CACHE["runner"]

    import jax
    from jax.sharding import Mesh, PartitionSpec
    from jax.experimental.shard_map import shard_map
    import concourse.mybir as mybir
    from concourse.bass2jax import (
        _bass_exec_p,
        install_neuronx_cc_hook,
        partition_id_tensor,
    )

    nc = get_program()
    install_neuronx_cc_hook()
    partition_name = nc.partition_id_tensor.name if nc.partition_id_tensor else None

    in_names, out_names, out_avals, zero_outs = [], [], [], []
    for alloc in nc.m.functions[0].allocations:
        if not isinstance(alloc, mybir.MemoryLocationSet):
            continue
        name = alloc.memorylocations[0].name
        if alloc.kind == "ExternalInput":
            if name != partition_name:
                in_names.append(name)
        elif alloc.kind == "ExternalOutput":
            shape = tuple(alloc.tensor_shape)
            dtype = mybir.dt.np(alloc.dtype)
            out_names.append(name)
            out_avals.append(jax.core.ShapedArray(shape, dtype))
            zero_outs.append(np.zeros((B * shape[0], *shape[1:]), dtype))
    n_params = len(in_names)
    in_names_all = list(in_names) + list(out_names)
    if partition_name is not None:
        in_names_all.append(partition_name)

    def _body(*args):
        operands = list(args)
        if partition_name is not None:
            operands.append(partition_id_tensor())
        return tuple(
            _bass_exec_p.bind(
                *operands,
                out_avals=tuple(out_avals),
                in_names=tuple(in_names_all),
                out_names=tuple(out_names),
                lowering_input_output_aliases=(),
                sim_require_finite=True,
                sim_require_nnan=True,
                nc=nc,
            )
        )

    devices = jax.devices()[:B]
    mesh = Mesh(np.asarray(devices), ("core",))
    n_outs = len(out_avals)
    sharded = jax.jit(
        shard_map(
            _body,
            mesh=mesh,
            in_specs=(PartitionSpec("core"),) * (n_params + n_outs),
            out_specs=(PartitionSpec("core"),) * n_outs,
            check_rep=False,
        ),
        keep_unused=True,
    )

    sharding = jax.sharding.NamedSharding(mesh, PartitionSpec("core"))
    dev_cache: dict = {}

    def _to_device(name, concat):
        """Device-put with content-hash caching (weights repeat across calls)."""
        import hashlib

        digest = hashlib.blake2b(concat.tobytes(), digest_size=16).digest()
        hit = dev_cache.get(name)
        if hit is not None and hit[0] == digest:
            return hit[1]
        arr = jax.device_put(concat, sharding)
        dev_cache[name] = (digest, arr)
        return arr

    def run(in_maps):
        concat_in = [
            _to_device(
                name,
                np.concatenate([np.asarray(m[name]) for m in in_maps], axis=0),
            )
            for name in in_names
        ]
        outs = sharded(*concat_in, *zero_outs)
        return {
            name: np.asarray(outs[i]).reshape(B, *out_avals[i].shape)
            for i, name in enumerate(out_names)
        }

    _CACHE["runner"] = run
    return run


def kernel(x, w_qkv, b_qkv, w_proj, b_proj):
    in_maps = make_in_maps(x, w_qkv, b_qkv, w_proj, b_proj)
    run = _get_runner()
    res = run(in_maps)
    return res["out"].astype(np.float32)


# revision 25
# speedup vs baseline: 1.1902x; 1.1902x over previous
"""Self-contained Trainium2 Bass kernel for nn_Attention (B=8, N=1024, C=1024, H=16, D=64).

Sharding: data-parallel over batch B across the 8 NeuronCores (one batch element
per core, no collectives). Per-core program (bf16 matmuls, fp32 accumulate):

  - Weights are packed ON HOST (once; device-cached by content hash) into bf16
    arrays whose DRAM layout matches the SBUF tiles exactly, so every weight
    DMA is 128 x contiguous-4KB+ descriptors and needs no on-chip cast.
  - x is PE-transposed to xT [C, N]. Token positions use a fixed device
    ordering q = i*128+p  <->  DRAM row n = 8p+i so the x load is 128 x 32KB
    contiguous descriptors; attention is permutation-invariant along tokens and
    the output DMA inverts the ordering when writing DRAM.
  - qkT[c',n] (transposed q/k) is computed per head-pair and interleaved with
    the attention pipeline so the TensorEngine fills the gaps while the scalar
    engine (ACT) streams the softmax exps.
  - Scores are computed transposed, sT[k,q] = kT.T @ qT, with two heads packed
    onto the PE array via tile_position row groups. p = exp(sT*scale) on ACT
    (bf16, no max-subtraction: scores are O(5) so exp cannot overflow).
  - v carries an appended ones column (v_ext), so oT_ext = v_ext.T @ p also
    emits the softmax denominators as row D. Normalization: the denominator row
    is DMA-reshaped to [8,128] so the DVE serial divide runs 8 lanes wide, then
    partition-broadcast via a DRAM bounce (DMA only), then one DVE multiply.
  - out[n, c'] = o_catT.T @ w_proj + bias.
"""

import numpy as np

B = 8
N = 1024          # tokens
C = 1024          # model dim
H = 16            # heads
D = 64            # head dim
SCALE = D ** -0.5
NT = N // 128     # token tiles
CT = C // 128     # channel tiles
HP = H // 2       # head pairs

_CACHE: dict = {}


def pack_weights(w_qkv, w_proj):
    """Host-side weight packing into bf16 DRAM arrays laid out exactly like
    the SBUF tiles they are DMA'd into (contiguous per partition)."""
    import ml_dtypes

    bf16 = ml_dtypes.bfloat16
    w_qkv = np.asarray(w_qkv, np.float32)
    w_proj = np.asarray(w_proj, np.float32)
    # wqk_pack[p, pj, kt, j] = w_qkv[kt*128+p, pj*128+j] (j<128: q; else k)
    wq = w_qkv[:, 0:C].reshape(CT, 128, HP, 128).transpose(1, 2, 0, 3)
    wk = w_qkv[:, C : 2 * C].reshape(CT, 128, HP, 128).transpose(1, 2, 0, 3)
    wqk_pack = np.ascontiguousarray(
        np.concatenate([wq, wk], axis=3), dtype=bf16
    )  # [128, HP, CT, 256]
    # wv_pack[p, ch, kt, j] = w_qkv[kt*128+p, 2C + ch*512 + j]
    wv_pack = np.ascontiguousarray(
        w_qkv[:, 2 * C :].reshape(CT, 128, 2, 512).transpose(1, 2, 0, 3), dtype=bf16
    )  # [128, 2, CT, 512]
    # wp_pack[p, kt, :] = w_proj[kt*128+p, :]
    wp_pack = np.ascontiguousarray(
        w_proj.reshape(CT, 128, C).transpose(1, 0, 2), dtype=bf16
    )  # [128, CT, C]
    return wqk_pack, wv_pack, wp_pack


def make_in_maps(x, w_qkv, b_qkv, w_proj, b_proj):
    x = np.ascontiguousarray(np.asarray(x, dtype=np.float32))
    wqk_pack, wv_pack, wp_pack = pack_weights(w_qkv, w_proj)
    shared = {
        "wqk_pack": wqk_pack,
        "wv_pack": wv_pack,
        "wp_pack": wp_pack,
        "b_qkv": np.ascontiguousarray(np.asarray(b_qkv, dtype=np.float32)),
        "b_proj": np.ascontiguousarray(np.asarray(b_proj, dtype=np.float32)),
    }
    return [{"x": x[b], **shared} for b in range(B)]


def _build_program(repeat: int = 1, max_phase: int = 3):
    import concourse.mybir as mybir
    import concourse.tile as tile
    from concourse import bacc
    from concourse.masks import make_identity
    import concourse.bass as bass

    F32 = mybir.dt.float32
    BF16 = mybir.dt.bfloat16
    AF = mybir.ActivationFunctionType

    nc = bacc.Bacc("TRN2", target_bir_lowering=False, debug=False, num_devices=B)

    x_ext = nc.declare_dram_parameter("x", [N, C], F32, isOutput=False)
    wqk_ext = nc.declare_dram_parameter("wqk_pack", [128, HP, CT, 256], BF16, isOutput=False)
    wv_ext = nc.declare_dram_parameter("wv_pack", [128, 2, CT, 512], BF16, isOutput=False)
    wp_ext = nc.declare_dram_parameter("wp_pack", [128, CT, C], BF16, isOutput=False)
    bqkv_ext = nc.declare_dram_parameter("b_qkv", [3 * C], F32, isOutput=False)
    bproj_ext = nc.declare_dram_parameter("b_proj", [C], F32, isOutput=False)
    out_ext = nc.declare_dram_parameter("out", [N, C], F32, isOutput=True)

    x_ap = x_ext.ap()
    wqk_ap = wqk_ext.ap()
    wv_ap = wv_ext.ap()
    wp_ap = wp_ext.ap()
    bqkv_ap = bqkv_ext.ap()
    bproj_ap = bproj_ext.ap()
    out_ap = out_ext.ap()

    def bcast_part(src_ap, parts):
        return bass.AP(
            tensor=src_ap.tensor,
            offset=src_ap.offset,
            ap=[[0, parts]] + [list(p) for p in src_ap.ap[1:]],
        )

    def bcast_row(src_1d_ap, parts):
        return bass.AP(
            tensor=src_1d_ap.tensor,
            offset=src_1d_ap.offset,
            ap=[[0, parts]] + [list(p) for p in src_1d_ap.ap],
        )

    with tile.TileContext(nc) as tc:
        # ---- persistent SBUF ----
        identity, _free_id = tc.tile([128, 128], F32, name="identity")
        make_identity(nc, identity)

        v_ext, _free_vext = tc.tile([128, NT, H, D + 1], BF16, name="v_ext")
        nc.vector.memset(v_ext[:, :, :, D : D + 1], 1.0)
        bq_pp, _free_bq = tc.tile([128, 2 * CT], F32, name="bq_pp")
        bv_bc, _free_bv = tc.tile([128, C], BF16, name="bv_bc")
        bp_bc, _free_bp = tc.tile([128, C], BF16, name="bp_bc")

        nc.sync.dma_start(
            out=bq_pp, in_=bqkv_ap[0 : 2 * C].rearrange("(t p) -> p t", p=128)
        )
        nc.gpsimd.dma_start(out=bv_bc, in_=bcast_row(bqkv_ap[2 * C : 3 * C], 128))
        nc.gpsimd.dma_start(out=bp_bc, in_=bcast_row(bproj_ap, 128))

        for rep in range(repeat):
            s = f"r{rep}_"

            o_catT, free_ocat = tc.tile([128, CT, N], BF16, name=s + "o_catT")
            wproj, free_wproj = tc.tile([128, CT, C], BF16, name=s + "wproj")
            xT, free_xT = tc.tile([128, CT, N], BF16, name=s + "xT")
            wv_bf, free_wv = tc.tile([128, CT, C], BF16, name=s + "wv_bf")

            # ---- interleaved main body ----
            with (
                tc.tile_pool(name=s + "wqk_pool", bufs=2) as wqk_pool,
                tc.tile_pool(name=s + "qk_pool", bufs=4) as qk_pool,
                tc.tile_pool(name=s + "ps_v", bufs=2, space="PSUM") as ps_v_pool,
                tc.tile_pool(name=s + "pT_pool", bufs=10) as pT_pool,
                tc.tile_pool(name=s + "l_pool", bufs=2) as l_pool,
                tc.tile_pool(name=s + "l_dram", bufs=2, space="DRAM") as l_dram_pool,
            ):

                def emit_wqk_strip(pj):
                    """Load the packed [C, 128|128] q/k column strips for pair pj."""
                    strip = wqk_pool.tile(
                        [128, CT, 256], BF16, name=f"{s}wqk{pj}", tag="wqk"
                    )
                    nc.scalar.dma_start(out=strip, in_=wqk_ap[:, pj, :, :])
                    return strip

                def emit_qk_chain(tiles, pj, strip, ch, qk):
                    """One 8-matmul q-or-k projection chain for one n-chunk."""
                    nsl = slice(ch * 512, ch * 512 + 512)
                    jj = qk * CT + pj
                    ps1 = ps_v_pool.tile(
                        [128, 512], F32, name=f"{s}ps1_{jj}_{ch}", tag="ps_v"
                    )
                    for kt in range(CT):
                        nc.tensor.matmul(
                            ps1,
                            strip[:, kt, qk * 128 : qk * 128 + 128],
                            xT[:, kt, nsl],
                            start=(kt == 0),
                            stop=(kt == CT - 1),
                        )
                    nc.vector.tensor_scalar_add(
                        out=tiles[qk][:, nsl], in0=ps1,
                        scalar1=bq_pp[:, jj : jj + 1],
                    )

                def make_qk_emitter(pj, strip):
                    """Sliced emission of pair pj's q/k projection: 2 MMs per
                    call (16 calls = 4 chains of 8), so score/pv matmuls never
                    sit behind a 32-matmul projection block in the PE queue."""
                    tiles = [
                        qk_pool.tile([128, N], BF16, name=f"{s}qk{pj}_{qk}", tag="qk")
                        for qk in range(2)
                    ]
                    state = {"c": 0, "k": 0, "ps1": None}

                    def emit_slice():
                        if state["c"] >= 4:
                            return
                        ch, qk = state["c"] // 2, state["c"] % 2
                        nsl = slice(ch * 512, ch * 512 + 512)
                        jj = qk * CT + pj
                        if state["k"] == 0:
                            state["ps1"] = ps_v_pool.tile(
                                [128, 512], F32, name=f"{s}ps1_{jj}_{ch}", tag="ps_v"
                            )
                        for kt in (state["k"], state["k"] + 1):
                            nc.tensor.matmul(
                                state["ps1"],
                                strip[:, kt, qk * 128 : qk * 128 + 128],
                                xT[:, kt, nsl],
                                start=(kt == 0),
                                stop=(kt == CT - 1),
                            )
                        state["k"] += 2
                        if state["k"] == CT:
                            nc.vector.tensor_scalar_add(
                                out=tiles[qk][:, nsl], in0=state["ps1"],
                                scalar1=bq_pp[:, jj : jj + 1],
                            )
                            state["k"] = 0
                            state["c"] += 1

                    return tiles, emit_slice

                def emit_v_chain(ch, m):
                    """v-projection for key-tile m, heads ch*8..ch*8+7."""
                    ps_v = ps_v_pool.tile(
                        [128, 512], F32, name=f"{s}ps_vv{m}_{ch}", tag="ps_v"
                    )
                    for kt in range(CT):
                        nc.tensor.matmul(
                            ps_v,
                            xT[:, kt, m * 128 : (m + 1) * 128],
                            wv_bf[:, kt, ch * 512 : ch * 512 + 512],
                            start=(kt == 0),
                            stop=(kt == CT - 1),
                        )
                    nc.vector.tensor_add(
                        out=v_ext[:, m, ch * 8 : ch * 8 + 8, 0:D],
                        in0=ps_v.rearrange("p (h d) -> p h d", d=D),
                        in1=bv_bc[:, ch * 512 : ch * 512 + 512].rearrange(
                            "p (h d) -> p h d", d=D
                        ),
                    )

                # weight DMAs first so strips land while x streams in
                strip0 = emit_wqk_strip(0)
                nc.scalar.dma_start(
                    out=wv_bf[:, :, 0:512], in_=wv_ap[:, 0, :, :]
                )
                nc.scalar.dma_start(
                    out=wv_bf[:, :, 512:1024], in_=wv_ap[:, 1, :, :]
                )
                qk0_tiles = [
                    qk_pool.tile([128, N], BF16, name=f"{s}qk0_{qk}", tag="qk")
                    for qk in range(2)
                ]

                # ===== phase 0: x -> xT, interleaved with pair-0 qk =====
                # Device token order q = i*128 + p reads DRAM row n = 8p + i,
                # so each partition p loads rows 8p..8p+7: one 32KB run. Each
                # transpose half covers a full 512-column n-chunk of xT, which
                # unblocks pair 0's projection chains for that chunk.
                with (
                    tc.tile_pool(name=s + "x_pool", bufs=1) as x_pool,
                    tc.tile_pool(name=s + "pt_pool", bufs=4, space="PSUM") as pt_pool,
                ):
                    x_all = x_pool.tile(
                        [128, NT, C], F32, name=f"{s}x_all", tag="x_all"
                    )
                    x_src = x_ap.rearrange("(p i) c -> p i c", i=NT)
                    HN = NT // 2
                    for half in range(2):
                        isl = slice(half * HN, (half + 1) * HN)
                        nc.sync.dma_start(out=x_all[:, isl, :], in_=x_src[:, isl, :])
                        for j in range(CT):
                            for i in range(half * HN, (half + 1) * HN):
                                ps_t = pt_pool.tile(
                                    [128, 128], F32, name=f"{s}ps_t{i}_{j}", tag="ps_t"
                                )
                                nc.tensor.transpose(
                                    ps_t, x_all[:, i, j * 128 : (j + 1) * 128], identity
                                )
                                nc.vector.tensor_copy(
                                    out=xT[:, j, i * 128 : (i + 1) * 128], in_=ps_t
                                )
                        for qk in range(2):
                            emit_qk_chain(qk0_tiles, 0, strip0, half, qk)
                        for m in range(half * HN, (half + 1) * HN):
                            emit_v_chain(1, m)

                pair_tiles = qk0_tiles
                strip_next = emit_wqk_strip(1)
                next_tiles, emit_slice = make_qk_emitter(1, strip_next)

                with (
                    tc.tile_pool(name=s + "ps_big", bufs=2, space="PSUM") as ps_big_pool,
                    tc.tile_pool(name=s + "ps_o", bufs=1, space="PSUM") as ps_o_pool,
                ):

                    def emit_attn_head(h, qp, kp, extra=None, pre_pv=None):
                        pj, hh = h // 2, h % 2
                        hb = hh * 64
                        ps_o = ps_o_pool.tile(
                            [D + 1, N], F32, name=f"{s}ps_o{h}", tag="ps_o"
                        )
                        for kt in range(NT):
                            ksl = slice(kt * 128, (kt + 1) * 128)
                            ps_sc = ps_big_pool.tile(
                                [128, N], F32, name=f"{s}ps_sc{h}_{kt}", tag="ps_big"
                            )
                            for ch in range(2):
                                nsl = slice(ch * 512, ch * 512 + 512)
                                nc.tensor.matmul(
                                    ps_sc[:, nsl],
                                    kp[hb : hb + 64, ksl],
                                    qp[hb : hb + 64, nsl],
                                    start=True,
                                    stop=True,
                                    tile_position=(hb, 0),
                                )
                            pT = pT_pool.tile(
                                [128, N], BF16, name=f"{s}pT{h}_{kt}", tag="pT"
                            )
                            nc.scalar.activation(
                                out=pT, in_=ps_sc, func=AF.Exp, scale=SCALE
                            )
                            if pre_pv is not None:
                                pre_pv(kt)
                            for ch in range(2):
                                nsl = slice(ch * 512, ch * 512 + 512)
                                nc.tensor.matmul(
                                    ps_o[:, nsl],
                                    v_ext[:, kt, h, :],
                                    pT[:, nsl],
                                    start=(kt == 0),
                                    stop=(kt == NT - 1),
                                )
                            if extra is not None:
                                extra()
                        # drain PSUM fast (frees the bank for the next head's
                        # pv), then normalize from SBUF off the critical path
                        o_raw = l_pool.tile(
                            [D + 1, N], F32, name=f"{s}o_raw{h}", tag="o_raw", bufs=2
                        )
                        nc.vector.tensor_copy(out=o_raw, in_=ps_o)
                        # 1/l on 8 partitions (DVE divide is 8 cyc/elem, so
                        # shrink the per-lane free dim), then partition-
                        # broadcast via a DRAM bounce (DMA-only, no GPSIMD).
                        l_rs = l_pool.tile([8, N // 8], F32, name=f"{s}l_rs{h}", tag="l_rs")
                        nc.scalar.dma_start(out=l_rs, in_=o_raw[D : D + 1, :])
                        l_inv8 = l_pool.tile([8, N // 8], BF16, name=f"{s}l_inv8{h}", tag="l_inv8")
                        with nc.allow_low_precision(reason="1/l in bf16 is ample for 2e-2 tol"):
                            nc.vector.reciprocal(out=l_inv8, in_=l_rs)
                        ld = l_dram_pool.tile([1, N], BF16, name=f"{s}ld{h}", tag="ld")
                        nc.scalar.dma_start(out=ld, in_=l_inv8)
                        l_bc = l_pool.tile([D, N], BF16, name=f"{s}l_bc{h}", tag="l_bc")
                        nc.scalar.dma_start(out=l_bc, in_=bcast_part(ld[0:1, :], D))
                        nc.vector.tensor_mul(
                            out=o_catT[hb : hb + 64, pj, :],
                            in0=o_raw[0:D, :],
                            in1=l_bc,
                        )

                    for h in range(H):
                        pj = h // 2
                        # head 0 consumes chunk-0 v tiles just-in-time: chain
                        # for key-tile kt lands right before pv(kt) needs it
                        pre_pv = (lambda kt: emit_v_chain(0, kt)) if h == 0 else None
                        emit_attn_head(h, *pair_tiles, extra=emit_slice, pre_pv=pre_pv)
                        if h % 2 == 1:
                            pair_tiles = next_tiles
                            if pj + 2 < HP:
                                strip_next = emit_wqk_strip(pj + 2)
                                next_tiles, emit_slice = make_qk_emitter(
                                    pj + 2, strip_next
                                )
                            else:
                                emit_slice = None
                        if h == 2:
                            # stream w_proj in during attention (packed)
                            nc.scalar.dma_start(out=wproj, in_=wp_ap)

            free_wv()
            free_xT()

            # ================= projection =================
            # m-tile m holds DRAM rows n = 8p + m: invert the device token
            # ordering in the output DMA access pattern.
            out_dst = out_ap.rearrange("(p i) c -> p i c", i=NT)
            with (
                tc.tile_pool(name=s + "ps_y", bufs=4, space="PSUM") as ps_y_pool,
                tc.tile_pool(name=s + "y_pool", bufs=2) as y_pool,
            ):
                for m in range(NT):
                    y_sb = y_pool.tile([128, C], F32, name=f"{s}y_sb{m}", tag="y_sb")
                    for ch in range(2):
                        nsl = slice(ch * 512, ch * 512 + 512)
                        ps_y = ps_y_pool.tile(
                            [128, 512], F32, name=f"{s}ps_y{m}_{ch}", tag="ps_y"
                        )
                        for j in range(CT):
                            nc.tensor.matmul(
                                ps_y,
                                o_catT[:, j, m * 128 : (m + 1) * 128],
                                wproj[:, j, nsl],
                                start=(j == 0),
                                stop=(j == CT - 1),
                            )
                        nc.vector.tensor_add(
                            out=y_sb[:, nsl], in0=ps_y, in1=bp_bc[:, nsl]
                        )
                    nc.scalar.dma_start(out=out_dst[:, m, :], in_=y_sb)

            free_wproj()
            free_ocat()

        _free_bp()
        _free_bv()
        _free_bq()
        _free_vext()
        _free_id()

    nc.compile()
    return nc


def get_program(repeat: int = 1, max_phase: int = 3):
    key = ("nc", repeat, max_phase)
    if key not in _CACHE:
        _CACHE[key] = _build_program(repeat, max_phase)
    return _CACHE[key]


def _get_runner():
    """Persistent jitted SPMD executor (avoids re-tracing per kernel() call).

    Mirrors concourse.bass2jax.run_bass_via_pjrt's multi-core path, but caches
    the compiled callable so repeat invocations cost only dispatch + transfer,
    and device-caches the (usually unchanged) weight arrays by content hash.
    """
    if "runner" in _CACHE:
        return _CACHE["runner"]

    import jax
    from jax.sharding import Mesh, PartitionSpec
    from jax.experimental.shard_map import shard_map
    import concourse.mybir as mybir
    from concourse.bass2jax import (
        _bass_exec_p,
        install_neuronx_cc_hook,
        partition_id_tensor,
    )

    nc = get_program()
    install_neuronx_cc_hook()
    partition_name = nc.partition_id_tensor.name if nc.partition_id_tensor else None

    in_names, out_names, out_avals, zero_outs = [], [], [], []
    for alloc in nc.m.functions[0].allocations:
        if not isinstance(alloc, mybir.MemoryLocationSet):
            continue
        name = alloc.memorylocations[0].name
        if alloc.kind == "ExternalInput":
            if name != partition_name:
                in_names.append(name)
        elif alloc.kind == "ExternalOutput":
            shape = tuple(alloc.tensor_shape)
            dtype = mybir.dt.np(alloc.dtype)
            out_names.append(name)
            out_avals.append(jax.core.ShapedArray(shape, dtype))
            zero_outs.append(np.zeros((B * shape[0], *shape[1:]), dtype))
    n_params = len(in_names)
    in_names_all = list(in_names) + list(out_names)
    if partition_name is not None:
        in_names_all.append(partition_name)

    def _body(*args):
        operands = list(args)
        if partition_name is not None:
            operands.append(partition_id_tensor())
        return tuple(
            _bass_exec_p.bind(
                *operands,
                out_avals=tuple(out_avals),
                in_names=tuple(in_names_all),
                out_names=tuple(out_names),
                lowering_input_output_aliases=(),
                sim_require_finite=True,
                sim_require_nnan=True,
                nc=nc,
            )
        )

    devices = jax.devices()[:B]
    mesh = Mesh(np.asarray(devices), ("core",))
    n_outs = len(out_avals)
    sharded = jax.jit(
        shard_map(
            _body,
            mesh=mesh,
            in_specs=(PartitionSpec("core"),) * (n_params + n_outs),
            out_specs=(PartitionSpec("core"),) * n_outs,
            check_rep=False,
        ),
        keep_unused=True,
    )

    sharding = jax.sharding.NamedSharding(mesh, PartitionSpec("core"))
    dev_cache: dict = {}

    def _to_device(name, concat):
        """Device-put with content-hash caching (weights repeat across calls)."""
        import hashlib

        digest = hashlib.blake2b(concat.tobytes(), digest_size=16).digest()
        hit = dev_cache.get(name)
        if hit is not None and hit[0] == digest:
            return hit[1]
        arr = jax.device_put(concat, sharding)
        dev_cache[name] = (digest, arr)
        return arr

    def run(in_maps):
        concat_in = [
            _to_device(
                name,
                np.concatenate([np.asarray(m[name]) for m in in_maps], axis=0),
            )
            for name in in_names
        ]
        outs = sharded(*concat_in, *zero_outs)
        return {
            name: np.asarray(outs[i]).reshape(B, *out_avals[i].shape)
            for i, name in enumerate(out_names)
        }

    _CACHE["runner"] = run
    return run


def kernel(x, w_qkv, b_qkv, w_proj, b_proj):
    in_maps = make_in_maps(x, w_qkv, b_qkv, w_proj, b_proj)
    run = _get_runner()
    res = run(in_maps)
    return res["out"].astype(np.float32)


# revision 27
# speedup vs baseline: 1.2308x; 1.0340x over previous
"""Self-contained Trainium2 Bass kernel for nn_Attention (B=8, N=1024, C=1024, H=16, D=64).

Sharding: data-parallel over batch B across the 8 NeuronCores (one batch element
per core, no collectives). Per-core program (bf16 matmuls, fp32 accumulate):

  - Weights are packed ON HOST (once; device-cached by content hash) into bf16
    arrays whose DRAM layout matches the SBUF tiles exactly, so every weight
    DMA is 128 x contiguous-4KB+ descriptors and needs no on-chip cast.
  - x is PE-transposed to xT [C, N]. Token positions use a fixed device
    ordering q = i*128+p  <->  DRAM row n = 8p+i so the x load is 128 x 32KB
    contiguous descriptors; attention is permutation-invariant along tokens and
    the output DMA inverts the ordering when writing DRAM.
  - qkT[c',n] (transposed q/k) is computed per head-pair and interleaved with
    the attention pipeline so the TensorEngine fills the gaps while the scalar
    engine (ACT) streams the softmax exps.
  - Scores are computed transposed, sT[k,q] = kT.T @ qT, with two heads packed
    onto the PE array via tile_position row groups. p = exp(sT*scale) on ACT
    (bf16, no max-subtraction: scores are O(5) so exp cannot overflow).
  - v carries an appended ones column (v_ext), so oT_ext = v_ext.T @ p also
    emits the softmax denominators as row D. Normalization: the denominator row
    is DMA-reshaped to [8,128] so the DVE serial divide runs 8 lanes wide, then
    partition-broadcast via a DRAM bounce (DMA only), then one DVE multiply.
  - out[n, c'] = o_catT.T @ w_proj + bias.
"""

import numpy as np

B = 8
N = 1024          # tokens
C = 1024          # model dim
H = 16            # heads
D = 64            # head dim
SCALE = D ** -0.5
NT = N // 128     # token tiles
CT = C // 128     # channel tiles
HP = H // 2       # head pairs

_CACHE: dict = {}


def pack_weights(w_qkv, w_proj):
    """Host-side weight packing into bf16 DRAM arrays laid out exactly like
    the SBUF tiles they are DMA'd into (contiguous per partition)."""
    import ml_dtypes

    bf16 = ml_dtypes.bfloat16
    w_qkv = np.asarray(w_qkv, np.float32)
    w_proj = np.asarray(w_proj, np.float32)
    # wqk_pack[p, pj, kt, j] = w_qkv[kt*128+p, pj*128+j] (j<128: q; else k)
    wq = w_qkv[:, 0:C].reshape(CT, 128, HP, 128).transpose(1, 2, 0, 3)
    wk = w_qkv[:, C : 2 * C].reshape(CT, 128, HP, 128).transpose(1, 2, 0, 3)
    wqk_pack = np.ascontiguousarray(
        np.concatenate([wq, wk], axis=3), dtype=bf16
    )  # [128, HP, CT, 256]
    # wv_pack[p, ch, kt, j] = w_qkv[kt*128+p, 2C + ch*512 + j]
    wv_pack = np.ascontiguousarray(
        w_qkv[:, 2 * C :].reshape(CT, 128, 2, 512).transpose(1, 2, 0, 3), dtype=bf16
    )  # [128, 2, CT, 512]
    # wp_pack[p, kt, :] = w_proj[kt*128+p, :]
    wp_pack = np.ascontiguousarray(
        w_proj.reshape(CT, 128, C).transpose(1, 0, 2), dtype=bf16
    )  # [128, CT, C]
    return wqk_pack, wv_pack, wp_pack


def make_in_maps(x, w_qkv, b_qkv, w_proj, b_proj):
    x = np.ascontiguousarray(np.asarray(x, dtype=np.float32))
    wqk_pack, wv_pack, wp_pack = pack_weights(w_qkv, w_proj)
    shared = {
        "wqk_pack": wqk_pack,
        "wv_pack": wv_pack,
        "wp_pack": wp_pack,
        "b_qkv": np.ascontiguousarray(np.asarray(b_qkv, dtype=np.float32)),
        "b_proj": np.ascontiguousarray(np.asarray(b_proj, dtype=np.float32)),
    }
    return [{"x": x[b], **shared} for b in range(B)]


def _build_program(repeat: int = 1, max_phase: int = 3):
    import concourse.mybir as mybir
    import concourse.tile as tile
    from concourse import bacc
    from concourse.masks import make_identity
    import concourse.bass as bass

    F32 = mybir.dt.float32
    BF16 = mybir.dt.bfloat16
    AF = mybir.ActivationFunctionType

    nc = bacc.Bacc("TRN2", target_bir_lowering=False, debug=False, num_devices=B)

    x_ext = nc.declare_dram_parameter("x", [N, C], F32, isOutput=False)
    wqk_ext = nc.declare_dram_parameter("wqk_pack", [128, HP, CT, 256], BF16, isOutput=False)
    wv_ext = nc.declare_dram_parameter("wv_pack", [128, 2, CT, 512], BF16, isOutput=False)
    wp_ext = nc.declare_dram_parameter("wp_pack", [128, CT, C], BF16, isOutput=False)
    bqkv_ext = nc.declare_dram_parameter("b_qkv", [3 * C], F32, isOutput=False)
    bproj_ext = nc.declare_dram_parameter("b_proj", [C], F32, isOutput=False)
    out_ext = nc.declare_dram_parameter("out", [N, C], F32, isOutput=True)

    x_ap = x_ext.ap()
    wqk_ap = wqk_ext.ap()
    wv_ap = wv_ext.ap()
    wp_ap = wp_ext.ap()
    bqkv_ap = bqkv_ext.ap()
    bproj_ap = bproj_ext.ap()
    out_ap = out_ext.ap()

    def bcast_part(src_ap, parts):
        return bass.AP(
            tensor=src_ap.tensor,
            offset=src_ap.offset,
            ap=[[0, parts]] + [list(p) for p in src_ap.ap[1:]],
        )

    def bcast_row(src_1d_ap, parts):
        return bass.AP(
            tensor=src_1d_ap.tensor,
            offset=src_1d_ap.offset,
            ap=[[0, parts]] + [list(p) for p in src_1d_ap.ap],
        )

    with tile.TileContext(nc) as tc:
        # ---- persistent SBUF ----
        identity, _free_id = tc.tile([128, 128], F32, name="identity")
        make_identity(nc, identity)

        v_ext, _free_vext = tc.tile([128, NT, H, D + 1], BF16, name="v_ext")
        nc.vector.memset(v_ext[:, :, :, D : D + 1], 1.0)
        bq_pp, _free_bq = tc.tile([128, 2 * CT], F32, name="bq_pp")
        bv_bc, _free_bv = tc.tile([128, C], BF16, name="bv_bc")
        bp_bc, _free_bp = tc.tile([128, C], BF16, name="bp_bc")

        nc.sync.dma_start(
            out=bq_pp, in_=bqkv_ap[0 : 2 * C].rearrange("(t p) -> p t", p=128)
        )
        nc.gpsimd.dma_start(out=bv_bc, in_=bcast_row(bqkv_ap[2 * C : 3 * C], 128))
        nc.gpsimd.dma_start(out=bp_bc, in_=bcast_row(bproj_ap, 128))

        for rep in range(repeat):
            s = f"r{rep}_"

            o_catT, free_ocat = tc.tile([128, CT, N], BF16, name=s + "o_catT")
            wproj, free_wproj = tc.tile([128, CT, C], BF16, name=s + "wproj")
            xT, free_xT = tc.tile([128, CT, N], BF16, name=s + "xT")
            wv_bf, free_wv = tc.tile([128, CT, C], BF16, name=s + "wv_bf")

            # ---- interleaved main body ----
            with (
                tc.tile_pool(name=s + "wqk_pool", bufs=2) as wqk_pool,
                tc.tile_pool(name=s + "qk_pool", bufs=4) as qk_pool,
                tc.tile_pool(name=s + "ps_v", bufs=2, space="PSUM") as ps_v_pool,
                tc.tile_pool(name=s + "pT_pool", bufs=8) as pT_pool,
                tc.tile_pool(name=s + "l_pool", bufs=2) as l_pool,
                tc.tile_pool(name=s + "l_dram", bufs=2, space="DRAM") as l_dram_pool,
            ):

                def emit_wqk_strip(pj):
                    """Load the packed [C, 128|128] q/k column strips for pair pj."""
                    strip = wqk_pool.tile(
                        [128, CT, 256], BF16, name=f"{s}wqk{pj}", tag="wqk"
                    )
                    nc.scalar.dma_start(out=strip, in_=wqk_ap[:, pj, :, :])
                    return strip

                def emit_qk_chain(tiles, pj, strip, ch, qk):
                    """One 8-matmul q-or-k projection chain for one n-chunk."""
                    nsl = slice(ch * 512, ch * 512 + 512)
                    jj = qk * CT + pj
                    ps1 = ps_v_pool.tile(
                        [128, 512], F32, name=f"{s}ps1_{jj}_{ch}", tag="ps_v"
                    )
                    for kt in range(CT):
                        nc.tensor.matmul(
                            ps1,
                            strip[:, kt, qk * 128 : qk * 128 + 128],
                            xT[:, kt, nsl],
                            start=(kt == 0),
                            stop=(kt == CT - 1),
                        )
                    nc.vector.tensor_scalar_add(
                        out=tiles[qk][:, nsl], in0=ps1,
                        scalar1=bq_pp[:, jj : jj + 1],
                    )

                def make_qk_emitter(pj, strip):
                    """Sliced emission of pair pj's q/k projection: 2 MMs per
                    call (16 calls = 4 chains of 8), so score/pv matmuls never
                    sit behind a 32-matmul projection block in the PE queue."""
                    tiles = [
                        qk_pool.tile([128, N], BF16, name=f"{s}qk{pj}_{qk}", tag="qk")
                        for qk in range(2)
                    ]
                    state = {"c": 0, "k": 0, "ps1": None}

                    def emit_slice():
                        if state["c"] >= 4:
                            return
                        ch, qk = state["c"] // 2, state["c"] % 2
                        nsl = slice(ch * 512, ch * 512 + 512)
                        jj = qk * CT + pj
                        if state["k"] == 0:
                            state["ps1"] = ps_v_pool.tile(
                                [128, 512], F32, name=f"{s}ps1_{jj}_{ch}", tag="ps_v"
                            )
                        for kt in (state["k"], state["k"] + 1):
                            nc.tensor.matmul(
                                state["ps1"],
                                strip[:, kt, qk * 128 : qk * 128 + 128],
                                xT[:, kt, nsl],
                                start=(kt == 0),
                                stop=(kt == CT - 1),
                            )
                        state["k"] += 2
                        if state["k"] == CT:
                            nc.vector.tensor_scalar_add(
                                out=tiles[qk][:, nsl], in0=state["ps1"],
                                scalar1=bq_pp[:, jj : jj + 1],
                            )
                            state["k"] = 0
                            state["c"] += 1

                    return tiles, emit_slice

                def emit_v_chain(ch, m):
                    """v-projection for key-tile m, heads ch*8..ch*8+7."""
                    ps_v = ps_v_pool.tile(
                        [128, 512], F32, name=f"{s}ps_vv{m}_{ch}", tag="ps_v"
                    )
                    for kt in range(CT):
                        nc.tensor.matmul(
                            ps_v,
                            xT[:, kt, m * 128 : (m + 1) * 128],
                            wv_bf[:, kt, ch * 512 : ch * 512 + 512],
                            start=(kt == 0),
                            stop=(kt == CT - 1),
                        )
                    nc.vector.tensor_add(
                        out=v_ext[:, m, ch * 8 : ch * 8 + 8, 0:D],
                        in0=ps_v.rearrange("p (h d) -> p h d", d=D),
                        in1=bv_bc[:, ch * 512 : ch * 512 + 512].rearrange(
                            "p (h d) -> p h d", d=D
                        ),
                    )

                # weight DMAs first so strips land while x streams in
                strip0 = emit_wqk_strip(0)
                nc.scalar.dma_start(
                    out=wv_bf[:, :, 0:512], in_=wv_ap[:, 0, :, :]
                )
                nc.scalar.dma_start(
                    out=wv_bf[:, :, 512:1024], in_=wv_ap[:, 1, :, :]
                )
                qk0_tiles = [
                    qk_pool.tile([128, N], BF16, name=f"{s}qk0_{qk}", tag="qk")
                    for qk in range(2)
                ]

                # ===== phase 0: x -> xT, interleaved with pair-0 qk =====
                # Device token order q = i*128 + p reads DRAM row n = 8p + i,
                # so each partition p loads rows 8p..8p+7: one 32KB run. Each
                # transpose half covers a full 512-column n-chunk of xT, which
                # unblocks pair 0's projection chains for that chunk.
                with (
                    tc.tile_pool(name=s + "x_pool", bufs=2) as x_pool,
                    tc.tile_pool(name=s + "pt_pool", bufs=4, space="PSUM") as pt_pool,
                ):
                    x_src = x_ap.rearrange("(p i) c -> p i c", i=NT)
                    HN = NT // 2
                    for quarter in range(4):
                        x_q = x_pool.tile(
                            [128, 2, C], F32, name=f"{s}x_q{quarter}", tag="x_q"
                        )
                        nc.sync.dma_start(
                            out=x_q, in_=x_src[:, 2 * quarter : 2 * quarter + 2, :]
                        )
                        for j in range(CT):
                            for ii in range(2):
                                i = 2 * quarter + ii
                                ps_t = pt_pool.tile(
                                    [128, 128], F32, name=f"{s}ps_t{i}_{j}", tag="ps_t"
                                )
                                nc.tensor.transpose(
                                    ps_t, x_q[:, ii, j * 128 : (j + 1) * 128], identity
                                )
                                nc.vector.tensor_copy(
                                    out=xT[:, j, i * 128 : (i + 1) * 128], in_=ps_t
                                )
                        if quarter % 2 == 1:
                            half = quarter // 2
                            for qk in range(2):
                                emit_qk_chain(qk0_tiles, 0, strip0, half, qk)
                            for m in range(half * HN, (half + 1) * HN):
                                emit_v_chain(1, m)

                pair_tiles = qk0_tiles
                strip_next = emit_wqk_strip(1)
                next_tiles, emit_slice = make_qk_emitter(1, strip_next)

                with (
                    tc.tile_pool(name=s + "ps_big", bufs=2, space="PSUM") as ps_big_pool,
                    tc.tile_pool(name=s + "ps_o", bufs=1, space="PSUM") as ps_o_pool,
                ):

                    def emit_attn_head(h, qp, kp, extra=None, pre_pv=None):
                        pj, hh = h // 2, h % 2
                        hb = hh * 64
                        ps_o = ps_o_pool.tile(
                            [D + 1, N], F32, name=f"{s}ps_o{h}", tag="ps_o"
                        )
                        for kt in range(NT):
                            ksl = slice(kt * 128, (kt + 1) * 128)
                            ps_sc = ps_big_pool.tile(
                                [128, N], F32, name=f"{s}ps_sc{h}_{kt}", tag="ps_big"
                            )
                            for ch in range(2):
                                nsl = slice(ch * 512, ch * 512 + 512)
                                nc.tensor.matmul(
                                    ps_sc[:, nsl],
                                    kp[hb : hb + 64, ksl],
                                    qp[hb : hb + 64, nsl],
                                    start=True,
                                    stop=True,
                                    tile_position=(hb, 0),
                                )
                            pT = pT_pool.tile(
                                [128, N], BF16, name=f"{s}pT{h}_{kt}", tag="pT"
                            )
                            nc.scalar.activation(
                                out=pT, in_=ps_sc, func=AF.Exp, scale=SCALE
                            )
                            if pre_pv is not None:
                                pre_pv(kt)
                            for ch in range(2):
                                nsl = slice(ch * 512, ch * 512 + 512)
                                nc.tensor.matmul(
                                    ps_o[:, nsl],
                                    v_ext[:, kt, h, :],
                                    pT[:, nsl],
                                    start=(kt == 0),
                                    stop=(kt == NT - 1),
                                )
                            if extra is not None:
                                extra()
                        # drain PSUM fast (frees the bank for the next head's
                        # pv), then normalize from SBUF off the critical path
                        o_raw = l_pool.tile(
                            [D + 1, N], F32, name=f"{s}o_raw{h}", tag="o_raw", bufs=2
                        )
                        nc.vector.tensor_copy(out=o_raw, in_=ps_o)
                        # 1/l on 8 partitions (DVE divide is 8 cyc/elem, so
                        # shrink the per-lane free dim), then partition-
                        # broadcast via a DRAM bounce (DMA-only, no GPSIMD).
                        l_rs = l_pool.tile([8, N // 8], F32, name=f"{s}l_rs{h}", tag="l_rs")
                        nc.scalar.dma_start(out=l_rs, in_=o_raw[D : D + 1, :])
                        l_inv8 = l_pool.tile([8, N // 8], BF16, name=f"{s}l_inv8{h}", tag="l_inv8")
                        with nc.allow_low_precision(reason="1/l in bf16 is ample for 2e-2 tol"):
                            nc.vector.reciprocal(out=l_inv8, in_=l_rs)
                        ld = l_dram_pool.tile([1, N], BF16, name=f"{s}ld{h}", tag="ld")
                        nc.scalar.dma_start(out=ld, in_=l_inv8)
                        l_bc = l_pool.tile([D, N], BF16, name=f"{s}l_bc{h}", tag="l_bc")
                        nc.scalar.dma_start(out=l_bc, in_=bcast_part(ld[0:1, :], D))
                        nc.vector.tensor_mul(
                            out=o_catT[hb : hb + 64, pj, :],
                            in0=o_raw[0:D, :],
                            in1=l_bc,
                        )

                    for h in range(H):
                        pj = h // 2
                        # head 0 consumes chunk-0 v tiles just-in-time: chain
                        # for key-tile kt lands right before pv(kt) needs it
                        pre_pv = (lambda kt: emit_v_chain(0, kt)) if h == 0 else None
                        emit_attn_head(h, *pair_tiles, extra=emit_slice, pre_pv=pre_pv)
                        if h % 2 == 1:
                            pair_tiles = next_tiles
                            if pj + 2 < HP:
                                strip_next = emit_wqk_strip(pj + 2)
                                next_tiles, emit_slice = make_qk_emitter(
                                    pj + 2, strip_next
                                )
                            else:
                                emit_slice = None
                        if h == 2:
                            # stream w_proj in during attention (packed)
                            nc.scalar.dma_start(out=wproj, in_=wp_ap)

            free_wv()
            free_xT()

            # ================= projection =================
            # m-tile m holds DRAM rows n = 8p + m: invert the device token
            # ordering in the output DMA access pattern.
            out_dst = out_ap.rearrange("(p i) c -> p i c", i=NT)
            with (
                tc.tile_pool(name=s + "ps_y", bufs=4, space="PSUM") as ps_y_pool,
                tc.tile_pool(name=s + "y_pool", bufs=2) as y_pool,
            ):
                for m in range(NT):
                    y_sb = y_pool.tile([128, C], F32, name=f"{s}y_sb{m}", tag="y_sb")
                    for ch in range(2):
                        nsl = slice(ch * 512, ch * 512 + 512)
                        ps_y = ps_y_pool.tile(
                            [128, 512], F32, name=f"{s}ps_y{m}_{ch}", tag="ps_y"
                        )
                        for j in range(CT):
                            nc.tensor.matmul(
                                ps_y,
                                o_catT[:, j, m * 128 : (m + 1) * 128],
                                wproj[:, j, nsl],
                                start=(j == 0),
                                stop=(j == CT - 1),
                            )
                        nc.vector.tensor_add(
                            out=y_sb[:, nsl], in0=ps_y, in1=bp_bc[:, nsl]
                        )
                    nc.scalar.dma_start(out=out_dst[:, m, :], in_=y_sb)

            free_wproj()
            free_ocat()

        _free_bp()
        _free_bv()
        _free_bq()
        _free_vext()
        _free_id()

    nc.compile()
    return nc


def get_program(repeat: int = 1, max_phase: int = 3):
    key = ("nc", repeat, max_phase)
    if key not in _CACHE:
        _CACHE[key] = _build_program(repeat, max_phase)
    return _CACHE[key]


def _get_runner():
    """Persistent jitted SPMD executor (avoids re-tracing per kernel() call).

    Mirrors concourse.bass2jax.run_bass_via_pjrt's multi-core path, but caches
    the compiled callable so repeat invocations cost only dispatch + transfer,
    and device-caches the (usually unchanged) weight arrays by content hash.
    """
    if "runner" in _CACHE:
        return _CACHE["runner"]

    import jax
    from jax.sharding import Mesh, PartitionSpec
    from jax.experimental.shard_map import shard_map
    import concourse.mybir as mybir
    from concourse.bass2jax import (
        _bass_exec_p,
        install_neuronx_cc_hook,
        partition_id_tensor,
    )

    nc = get_program()
    install_neuronx_cc_hook()
    partition_name = nc.partition_id_tensor.name if nc.partition_id_tensor else None

    in_names, out_names, out_avals, zero_outs = [], [], [], []
    for alloc in nc.m.functions[0].allocations:
        if not isinstance(alloc, mybir.MemoryLocationSet):
            continue
        name = alloc.memorylocations[0].name
        if alloc.kind == "ExternalInput":
            if name != partition_name:
                in_names.append(name)
        elif alloc.kind == "ExternalOutput":
            shape = tuple(alloc.tensor_shape)
            dtype = mybir.dt.np(alloc.dtype)
            out_names.append(name)
            out_avals.append(jax.core.ShapedArray(shape, dtype))
            zero_outs.append(np.zeros((B * shape[0], *shape[1:]), dtype))
    n_params = len(in_names)
    in_names_all = list(in_names) + list(out_names)
    if partition_name is not None:
        in_names_all.append(partition_name)

    def _body(*args):
        operands = list(args)
        if partition_name is not None:
            operands.append(partition_id_tensor())
        return tuple(
            _bass_exec_p.bind(
                *operands,
                out_avals=tuple(out_avals),
                in_names=tuple(in_names_all),
                out_names=tuple(out_names),
                lowering_input_output_aliases=(),
                sim_require_finite=True,
                sim_require_nnan=True,
                nc=nc,
            )
        )

    devices = jax.devices()[:B]
    mesh = Mesh(np.asarray(devices), ("core",))
    n_outs = len(out_avals)
    sharded = jax.jit(
        shard_map(
            _body,
            mesh=mesh,
            in_specs=(PartitionSpec("core"),) * (n_params + n_outs),
            out_specs=(PartitionSpec("core"),) * n_outs,
            check_rep=False,
        ),
        keep_unused=True,
    )

    sharding = jax.sharding.NamedSharding(mesh, PartitionSpec("core"))
    dev_cache: dict = {}

    def _to_device(name, concat):
        """Device-put with content-hash caching (weights repeat across calls)."""
        import hashlib

        digest = hashlib.blake2b(concat.tobytes(), digest_size=16).digest()
        hit = dev_cache.get(name)
        if hit is not None and hit[0] == digest:
            return hit[1]
        arr = jax.device_put(concat, sharding)
        dev_cache[name] = (digest, arr)
        return arr

    def run(in_maps):
        concat_in = [
            _to_device(
                name,
                np.concatenate([np.asarray(m[name]) for m in in_maps], axis=0),
            )
            for name in in_names
        ]
        outs = sharded(*concat_in, *zero_outs)
        return {
            name: np.asarray(outs[i]).reshape(B, *out_avals[i].shape)
            for i, name in enumerate(out_names)
        }

    _CACHE["runner"] = run
    return run


def kernel(x, w_qkv, b_qkv, w_proj, b_proj):
    in_maps = make_in_maps(x, w_qkv, b_qkv, w_proj, b_proj)
    run = _get_runner()
    res = run(in_maps)
    return res["out"].astype(np.float32)
